# revision 25
# baseline (speedup 1.0000x reference)
"""Class-balanced softmax cross-entropy loss on 8 Trainium2 NeuronCores.

Math: counts N_c over batch; w_c = (1-beta)/(1-beta^N_c) (0 if N_c=0);
loss = -sum w[t](logp[t]) / sum w[t] over valid pixels.

Fast path (used when all class weights are equal, which holds whenever every
class count N_c is large enough that beta^N_c underflows — always true for
this problem's 4.2M uniformly distributed pixels; verified exactly on host
via bincount): the weights cancel in the ratio, so
  loss = (sum_pix lse - sum_pix x[t]) / N_valid

fast3 (no-ignore) engine split per core (91.8us HW, vs 153.1us baseline):
  ACT : exp over all logits in DMA-slab-sized instrs (the ~65us/core floor;
        slab-granular waits keep the stream dense behind the DMA), writing
        fp8e4 E; per-chunk Ln(sumexp) from PSUM with accum_out -> Σ lse.
        A dummy activation up front pulls the ~1.3us ACT table load into
        the DMA-fill window.
  PE  : sumexp = Σ_c exp via fp8 DoubleRow matmuls (2 classes/matmul, PSUM
        accumulation; replaces the old DVE f16 tree), and Σ x[t] = Σ of
        mask*x products via ones-matmuls into one accumulating PSUM bank.
  DVE : per-class one-hot masks via tensor_scalar is_equal (4x mode) plus
        one in-place tensor_tensor product per class-group (2x mode); the
        old fused scalar_tensor_tensor path has no fast uops (1x) and was
        the 153us bottleneck. Final ps_g -> scalar reduce also on DVE.
  Out : the [128, 8] ABN accumulator (4 Ln cols + gather total) is DMA'd
        straight to DRAM; the host does the final partition reduce.
Inputs host-cast: logits -> bf16 chunk-major [4*128, 19456], target -> f16.
Pipeline: 4 col-chunks x ~5-class groups, X/E double-buffered, 4 rotating
mask buffers; chunk 0 split finer for fill, chunk 3 ends with tiny slabs
and the PE runs gathers before sums there to shorten the tail.

Exact fallback path (any weight spread): original per-class A/B/N kernel;
masked stt path for inputs with ignore pixels.
"""

import numpy as np
import sys

for _p in ("/opt/trn_rl_repo",):
    if _p not in sys.path:
        sys.path.insert(0, _p)

import ml_dtypes
from concourse import bass, mybir
from concourse.bass_utils import run_bass_kernel_spmd

NCLASS = 19
BETA = 0.999
NCORES = 8
P = 128
COLS = 4096              # 512*1024 / 128
F = 1024                 # free-dim chunk
NCHUNK = COLS // F       # 4
EF = NCLASS * F          # 19456
GROUPS = [(0, 5), (5, 10), (10, 15), (15, 19)]
GW = 5 * F               # max group width in cols

f32 = mybir.dt.float32
f16 = mybir.dt.float16
bf16 = mybir.dt.bfloat16
i32 = mybir.dt.int32
AF = mybir.ActivationFunctionType
ALU = mybir.AluOpType


def _build_fast3():
    f8 = mybir.dt.float8e4
    nc = bass.Bass()
    xp = nc.declare_dram_parameter("xp", [NCHUNK * P, EF], bf16, isOutput=False)
    tgt = nc.declare_dram_parameter("tgt", [P, COLS], f16, isOutput=False)
    w8_in = nc.declare_dram_parameter("w8", [P, 2 * P], f8, isOutput=False)
    out = nc.declare_dram_parameter("out", [1, 8], f32, isOutput=True)

    NMBUF = 4                # rotating mask/product buffers
    X2 = nc.alloc_sbuf_tensor("X2", [P, 2 * EF], bf16)
    E2 = nc.alloc_sbuf_tensor("E2", [P, 2 * EF], f8)
    T = nc.alloc_sbuf_tensor("T", [P, COLS], f16)
    M2 = nc.alloc_sbuf_tensor("M2", [P, NMBUF * GW], f16)
    junkL = nc.alloc_sbuf_tensor("junkL", [P, F], f16)
    ABN = nc.alloc_sbuf_tensor("ABN", [P, 8], f32)
    W8 = nc.alloc_sbuf_tensor("W8", [P, 2 * P], f8)
    ones_g = nc.alloc_sbuf_tensor("ones_g", [P, 1], f16)
    ones_f = nc.alloc_sbuf_tensor("ones_f", [P, 1], f32)
    junkG = nc.alloc_sbuf_tensor("junkG", [1, 512], f16)
    res = nc.alloc_sbuf_tensor("res", [1, 8], f32)
    ps_s = nc.alloc_psum_tensor("ps_s", [P, 2 * F], f32)
    ps_g = nc.alloc_psum_tensor("ps_g", [1, 512], f32)
    ps2 = nc.alloc_psum_tensor("ps2", [1, 8], f32)

    # sumexp class pairs (DoubleRow sums 2 classes/matmul); grouped so waits
    # track exp progress
    PAIR_GROUPS = [[(0, 1), (2, 3)], [(4, 5), (6, 7), (8, 9)],
                   [(10, 11), (12, 13)], [(14, 15), (16, 17)], [(18, None)]]
    # per-chunk X sub-DMA class splits (chunk 0 finer for fill, chunk 3 with a
    # tiny last slab for the tail); exp instrs mirror slabs except middle
    # chunks which batch up
    DMA_SPLITS = [[(1, 3), (3, 5), (5, 10), (10, 15), (15, 19)],
                  [(0, 5), (5, 10), (10, 15), (15, 19)],
                  [(0, 5), (5, 10), (10, 15), (15, 19)],
                  [(0, 5), (5, 10), (10, 15), (15, 18), (18, 19)]]
    EXP_SPLITS = [[(0, 1)] + DMA_SPLITS[0],
                  [(0, 10), (10, 19)], [(0, 10), (10, 19)],
                  DMA_SPLITS[3]]
    # per-chunk mask/product groups (chunk 3 ends with a 2-class group)
    MGROUPS = [GROUPS, GROUPS, GROUPS,
               [(0, 5), (5, 10), (10, 15), (15, 17), (17, 19)]]
    mg_cum = [0]
    for k in range(NCHUNK):
        mg_cum.append(mg_cum[-1] + len(MGROUPS[k]))
    NMG = mg_cum[-1]

    dma_done_at = []
    n = 0
    for k in range(NCHUNK):
        ends = {}
        for (lo, hi) in DMA_SPLITS[k]:
            n += 1
            ends[hi] = n
        dma_done_at.append(ends)

    def xdma_thr(k, hi):
        ends = dma_done_at[k]
        best = min(e for e in ends if e >= hi)
        return 16 * ends[best]

    def wait_x(eng, k, hi):
        if k == 0 and hi <= 1:
            eng.wait_ge(s_x1, 16)
            return
        if k == 0:
            eng.wait_ge(s_x1, 16)
        eng.wait_ge(s_x, xdma_thr(k, hi))

    def tt_thr(k, hi):
        """s_tt threshold for products covering classes [0, hi) of chunk k."""
        ng = 0
        for (lo2, hi2) in MGROUPS[k]:
            ng += 1
            if hi2 >= hi:
                break
        return mg_cum[k] + ng

    with (
        nc.Block() as block,
        nc.semaphore("s_t") as s_t,
        nc.semaphore("s_id") as s_id,
        nc.semaphore("s_x") as s_x,
        nc.semaphore("s_x1") as s_x1,
        nc.semaphore("s_fin") as s_fin,
        nc.semaphore("s_e") as s_e,      # counts classes exp'd: 19*k + hi
        nc.semaphore("s_tt") as s_tt,
        nc.semaphore("s_ps") as s_ps,
        nc.semaphore("s_pg") as s_pg,
        nc.semaphore("s_ln") as s_ln,
        nc.semaphore("s_gs") as s_gs,
        nc.semaphore("s_out") as s_out,
        nc.allow_low_precision("f16 masks/products; fp8 exp; f32 accum"),
    ):
        @block.sync
        def _(sp):
            for k in range(NCHUNK):
                h = k % 2
                for j, (lo, hi) in enumerate(DMA_SPLITS[k]):
                    if k >= 2:
                        # X half reused: chunk k-2's exp + products done
                        sp.wait_ge(s_e, 19 * (k - 2) + hi)
                        sp.wait_ge(s_tt, tt_thr(k - 2, hi))
                    sp.dma_start(
                        X2[:, h * EF + lo * F: h * EF + hi * F],
                        xp[k * P:(k + 1) * P, lo * F: hi * F],
                    ).then_inc(s_x, 16)
                    if k == 0 and j == 0:
                        sp.dma_start(T[:, 0:F], tgt[:, 0:F]).then_inc(s_t, 16)
                        sp.dma_start(W8[:], w8_in[:, :]).then_inc(s_id, 16)
                    elif k >= 1 and j == 0:
                        sp.dma_start(T[:, k * F:(k + 1) * F],
                                     tgt[:, k * F:(k + 1) * F]).then_inc(s_t, 16)

        @block.scalar
        def _(act):
            def ln_chunk(kk):
                hh = kk % 2
                act.wait_ge(s_ps, kk + 1)
                act.activation(
                    junkL[:], ps_s[:, hh * F:(hh + 1) * F], AF.Ln,
                    accum_out=ABN[:, kk:kk + 1]).then_inc(s_ln, 1)

            # first X slab rides ACT's own HWDGE queue, in parallel with
            # the SP DMA stream; the dummy activation pulls the ACT table
            # load into the DMA-fill window
            act.dma_start(X2[:, 0:F], xp[0:P, 0:F]).then_inc(s_x1, 16)
            act.activation(junkL[:, 0:16], junkL[:, 16:32], AF.Exp)
            for k in range(NCHUNK):
                h = k % 2
                ln_at = 0 if len(EXP_SPLITS[k]) == 2 else 1
                for j, (lo, hi) in enumerate(EXP_SPLITS[k]):
                    wait_x(act, k, hi)
                    if k >= 2 and j == 0:
                        act.wait_ge(s_ps, k - 1)   # E half reused
                    act.activation(
                        E2[:, h * EF + lo * F: h * EF + hi * F],
                        X2[:, h * EF + lo * F: h * EF + hi * F],
                        AF.Exp).then_inc(s_e, hi - lo)
                    if k >= 1 and j == ln_at:
                        ln_chunk(k - 1)
            ln_chunk(NCHUNK - 1)
            # tail: partition-reduce on PE, then one 32B-descriptor DMA out
            # (a [128,8] DMA is 128 sub-512B descriptors and costs ~7us)
            act.wait_ge(s_fin, 1)
            act.copy(res[:], ps2[:])
            act.dma_start(out[:, :], res[:]).then_inc(s_out, 16)
            act.wait_ge(s_out, 16)

        @block.vector
        def _(dve):
            dve.memset(ABN[:], 0.0)
            dve.memset(ones_g[:], 1.0)
            dve.memset(ones_f[:], 1.0)
            for k in range(NCHUNK):
                h = k % 2
                dve.wait_ge(s_t, 16 * (k + 1))
                Tk = T[:, k * F:(k + 1) * F]
                for g, (lo, hi) in enumerate(MGROUPS[k]):
                    G = mg_cum[k] + g
                    hm = G % NMBUF
                    W = (hi - lo) * F
                    Mg = M2[:, hm * GW: hm * GW + W]
                    if G >= NMBUF:
                        dve.wait_ge(s_pg, G - (NMBUF - 1))   # M buf reused
                    for ci, c in enumerate(range(lo, hi)):
                        dve.tensor_scalar(
                            out=Mg[:, ci * F:(ci + 1) * F], in0=Tk,
                            scalar1=float(c), scalar2=None, op0=ALU.is_equal)
                    wait_x(dve, k, hi)
                    dve.tensor_tensor(
                        out=Mg[:], in0=Mg[:],
                        in1=X2[:, h * EF + lo * F: h * EF + hi * F],
                        op=ALU.mult).then_inc(s_tt, 1)
            # total gather sum: reduce ps_g into ABN[0, 4] (rows 1.. are 0)
            dve.wait_ge(s_pg, NMG)
            dve.tensor_scalar(
                out=junkG[:], in0=ps_g[:, :], scalar1=1.0, scalar2=None,
                op0=ALU.mult, op1=ALU.add,
                accum_out=ABN[0:1, 4:5]).then_inc(s_gs, 1)

        @block.tensor
        def _(pe):
            pe.wait_ge(s_id, 16)
            w_dr = W8[:].rearrange("p (two m) -> p two m", two=2)
            first_g = True

            def sum_pairs(k, pg):
                h = k % 2
                for (c0, c1) in PAIR_GROUPS[pg]:
                    pe.wait_ge(s_e, 19 * k + (c1 if c1 is not None else c0) + 1)
                    for half in range(2):
                        dst = ps_s[:, h * F + half * 512: h * F + (half + 1) * 512]
                        base = h * EF + c0 * F + half * 512
                        if c1 is not None:
                            rhs = E2[:, base: base + 2 * F].rearrange(
                                "p (two f) -> p two f", two=2)[:, :, 0:512]
                            pe.matmul(
                                dst, lhsT=w_dr, rhs=rhs,
                                perf_mode=mybir.MatmulPerfMode.DoubleRow,
                                start=(c0 == 0), stop=False,
                                skip_group_check=True)
                        else:
                            ins = pe.matmul(
                                dst, lhsT=W8[:, 0:P],
                                rhs=E2[:, base: base + 512],
                                start=False, stop=True,
                                skip_group_check=True)
                            if half == 1:
                                ins.then_inc(s_ps, 1)

            def gather(k, g):
                nonlocal first_g
                lo, hi = MGROUPS[k][g]
                G = mg_cum[k] + g
                hm = G % NMBUF
                pe.wait_ge(s_tt, G + 1)
                n = (hi - lo) * 2
                for j in range(n):
                    ins = pe.matmul(
                        ps_g[:, :], lhsT=ones_g[:],
                        rhs=M2[:, hm * GW + j * 512: hm * GW + (j + 1) * 512],
                        start=first_g, stop=(G == NMG - 1 and j == n - 1),
                        skip_group_check=True)
                    first_g = False
                    if j == n - 1:
                        ins.then_inc(s_pg, 1)

            for k in range(NCHUNK):
                if k >= 2:
                    pe.wait_ge(s_ln, k - 1)        # psum half reused
                if k < NCHUNK - 1:
                    npg = len(PAIR_GROUPS)
                    nmg = len(MGROUPS[k])
                    for i in range(max(npg, nmg)):
                        if i < npg:
                            sum_pairs(k, i)
                        if i < nmg:
                            gather(k, i)
                else:
                    order = [("g", 0), ("s", 0), ("g", 1), ("s", 1),
                             ("g", 2), ("s", 2), ("g", 3), ("s", 3),
                             ("g", 4), ("s", 4)]
                    for kind, i in order:
                        if kind == "g":
                            gather(k, i)
                        else:
                            sum_pairs(k, i)
            pe.wait_ge(s_ln, NCHUNK)
            pe.wait_ge(s_gs, 1)
            pe.matmul(ps2[:], lhsT=ones_f[:], rhs=ABN[:],
                      start=True, stop=True,
                      skip_group_check=True).then_inc(s_fin, 1)

    return nc


def _build_fast():
    """Masked fast path (handles ignore pixels); original stt kernel."""
    nc = bass.Bass()
    logits = nc.declare_dram_parameter("logits", [NCLASS, P, COLS], bf16, isOutput=False)
    target = nc.declare_dram_parameter("target", [P, COLS], f32, isOutput=False)
    NG = NCHUNK * NCLASS
    ACC_COLS = NG + NCHUNK
    out = nc.declare_dram_parameter("out", [1, ACC_COLS], f32, isOutput=True)

    X2 = nc.alloc_sbuf_tensor("X2", [P, 2 * EF], bf16)
    E2 = nc.alloc_sbuf_tensor("E2", [P, 2 * EF], f16)
    T = nc.alloc_sbuf_tensor("T", [P, COLS], f32)
    L2 = nc.alloc_sbuf_tensor("L2", [P, 2 * F], f16)
    junk = nc.alloc_sbuf_tensor("junk", [P, F], f16)
    ABN = nc.alloc_sbuf_tensor("ABN", [P, ACC_COLS], f32)
    ones = nc.alloc_sbuf_tensor("ones", [P, 1], f32)
    res = nc.alloc_sbuf_tensor("res", [1, ACC_COLS], f32)
    ps = nc.alloc_psum_tensor("ps", [1, ACC_COLS], f32)

    with (
        nc.Block() as block,
        nc.semaphore("s_x") as s_x,
        nc.semaphore("s_x1") as s_x1,
        nc.semaphore("s_fin") as s_fin,
        nc.semaphore("s_t") as s_t,
        nc.semaphore("s_exp") as s_exp,
        nc.semaphore("s_tree") as s_tree,
        nc.semaphore("s_log") as s_log,
        nc.semaphore("s_gA") as s_gA,
        nc.semaphore("s_gB") as s_gB,
        nc.semaphore("s_mm") as s_mm,
        nc.semaphore("s_out") as s_out,
        nc.allow_low_precision("f16 tree-sum of exp; error ~0.1% on lse"),
    ):
        @block.sync
        def _(sp):
            sp.dma_start(T[:], target[:, :]).then_inc(s_t, 16)
            for k in range(NCHUNK):
                h = k % 2
                if k >= 2:
                    sp.wait_ge(s_gA, k - 1)
                sp.dma_start(
                    X2[:, h * EF:(h + 1) * EF].rearrange("p (c f) -> p c f", c=NCLASS),
                    logits[:, :, k * F:(k + 1) * F].rearrange("c p f -> p c f"),
                ).then_inc(s_x, 16)

        @block.scalar
        def _(act):
            for k in range(NCHUNK):
                h = k % 2
                act.wait_ge(s_x, 16 * (k + 1))
                E = E2[:, h * EF:(h + 1) * EF]
                X = X2[:, h * EF:(h + 1) * EF]
                for c in range(NCLASS):
                    ins = act.activation(
                        E[:, c * F:(c + 1) * F], X[:, c * F:(c + 1) * F], AF.Exp)
                    if c == NCLASS - 1:
                        ins.then_inc(s_exp, 1)
                act.wait_ge(s_tree, k + 1)
                if k >= 2:
                    act.wait_ge(s_gB, k - 1)   # L half reused
                act.activation(
                    L2[:, h * F:(h + 1) * F], E[:, 0:F], AF.Ln,
                ).then_inc(s_log, 1)
            act.wait_ge(s_mm, 1)
            act.copy(res[:], ps[:])
            act.dma_start(out[:, :], res[:]).then_inc(s_out, 16)
            act.wait_ge(s_out, 16)

        @block.vector
        def _(dve):
            dve.memset(ABN[:], 0.0)
            dve.memset(ones[:], 1.0)
            dve.wait_ge(s_t, 16)
            for k in range(NCHUNK):
                h = k % 2
                dve.wait_ge(s_exp, k + 1)
                E = E2[:, h * EF:(h + 1) * EF]
                dve.tensor_tensor(out=E[:, 0:3 * F], in0=E[:, 0:3 * F],
                                  in1=E[:, 16 * F:19 * F], op=ALU.add)
                dve.tensor_tensor(out=E[:, 0:8 * F], in0=E[:, 0:8 * F],
                                  in1=E[:, 8 * F:16 * F], op=ALU.add)
                dve.tensor_tensor(out=E[:, 0:4 * F], in0=E[:, 0:4 * F],
                                  in1=E[:, 4 * F:8 * F], op=ALU.add)
                dve.tensor_tensor(out=E[:, 0:2 * F], in0=E[:, 0:2 * F],
                                  in1=E[:, 2 * F:4 * F], op=ALU.add)
                dve.tensor_tensor(out=E[:, 0:F], in0=E[:, 0:F],
                                  in1=E[:, F:2 * F], op=ALU.add).then_inc(s_tree, 1)
                X = X2[:, h * EF:(h + 1) * EF]
                Tk = T[:, k * F:(k + 1) * F]
                for c in range(NCLASS):
                    ins = dve.scalar_tensor_tensor(
                        out=junk[:], in0=Tk, scalar=float(c),
                        in1=X[:, c * F:(c + 1) * F],
                        op0=ALU.is_equal, op1=ALU.mult,
                        accum_out=ABN[:, k * NCLASS + c: k * NCLASS + c + 1])
                    if c == NCLASS - 1:
                        ins.then_inc(s_gA, 1)
                dve.wait_ge(s_log, k + 1)
                dve.scalar_tensor_tensor(
                    out=junk[:], in0=Tk, scalar=-0.5,
                    in1=L2[:, h * F:(h + 1) * F],
                    op0=ALU.is_gt, op1=ALU.mult,
                    accum_out=ABN[:, NG + k: NG + k + 1]).then_inc(s_gB, 1)

        @block.tensor
        def _(pe):
            pe.wait_ge(s_gB, NCHUNK)
            pe.matmul(ps[:], lhsT=ones[:], rhs=ABN[:], start=True, stop=True
                      ).then_inc(s_mm, 1)

    return nc


def _build_exact():
    """Original per-class A/B/N kernel (correct for any weight pattern)."""
    nc = bass.Bass()
    F0 = 512
    NCH0 = COLS // F0
    SEC = NCH0 * NCLASS
    ACC0 = 3 * SEC
    logits = nc.declare_dram_parameter("logits", [NCLASS, P, COLS], f32, isOutput=False)
    target = nc.declare_dram_parameter("target", [P, COLS], i32, isOutput=False)
    out = nc.declare_dram_parameter("out", [1, ACC0], f32, isOutput=True)

    EF0 = NCLASS * F0
    X2 = nc.alloc_sbuf_tensor("X2", [P, 2 * EF0], f32)
    E2 = nc.alloc_sbuf_tensor("E2", [P, 2 * EF0], f32)
    Ti2 = nc.alloc_sbuf_tensor("Ti2", [P, 2 * F0], i32)
    Tf2 = nc.alloc_sbuf_tensor("Tf2", [P, 2 * F0], f32)
    S2 = nc.alloc_sbuf_tensor("S2", [P, 2 * F0], f32)
    L2 = nc.alloc_sbuf_tensor("L2", [P, 2 * F0], f32)
    junk = nc.alloc_sbuf_tensor("junk", [P, F0], f32)
    ABN = nc.alloc_sbuf_tensor("ABN", [P, ACC0], f32)
    ones = nc.alloc_sbuf_tensor("ones", [P, 1], f32)
    res = nc.alloc_sbuf_tensor("res", [1, ACC0], f32)
    ps = nc.alloc_psum_tensor("ps", [1, ACC0], f32)

    with (
        nc.Block() as block,
        nc.semaphore("sem_x") as sem_x,
        nc.semaphore("sem_t") as sem_t,
        nc.semaphore("sem_exp") as sem_exp,
        nc.semaphore("sem_red") as sem_red,
        nc.semaphore("sem_log") as sem_log,
        nc.semaphore("sem_done") as sem_done,
        nc.semaphore("sem_mm") as sem_mm,
        nc.semaphore("sem_out") as sem_out,
    ):
        @block.scalar
        def _(act):
            for k in range(NCH0):
                h = k % 2
                if k >= 2:
                    act.wait_ge(sem_done, k - 1)
                act.dma_start(
                    X2[:, h * EF0:(h + 1) * EF0].rearrange("p (c f) -> p c f", c=NCLASS),
                    logits[:, :, k * F0:(k + 1) * F0].rearrange("c p f -> p c f"),
                ).then_inc(sem_x, 16)
                act.dma_start(
                    Ti2[:, h * F0:(h + 1) * F0], target[:, k * F0:(k + 1) * F0],
                ).then_inc(sem_t, 16)
                act.wait_ge(sem_x, 16 * (k + 1))
                for c in range(NCLASS):
                    ins = act.activation(
                        E2[:, h * EF0 + c * F0: h * EF0 + (c + 1) * F0],
                        X2[:, h * EF0 + c * F0: h * EF0 + (c + 1) * F0], AF.Exp)
                    if c == NCLASS - 1:
                        ins.then_inc(sem_exp, 1)
                act.wait_ge(sem_red, k + 1)
                act.activation(
                    L2[:, h * F0:(h + 1) * F0], S2[:, h * F0:(h + 1) * F0], AF.Ln,
                ).then_inc(sem_log, 1)
            act.wait_ge(sem_mm, 1)
            act.copy(res[:], ps[:])
            act.dma_start(out[:, :], res[:]).then_inc(sem_out, 16)
            act.wait_ge(sem_out, 16)

        @block.vector
        def _(dve):
            dve.memset(ABN[:], 0.0)
            dve.memset(ones[:], 1.0)
            for k in range(NCH0):
                h = k % 2
                dve.wait_ge(sem_exp, k + 1)
                dve.tensor_reduce(
                    S2[:, h * F0:(h + 1) * F0],
                    E2[:, h * EF0:(h + 1) * EF0].rearrange("p (c f) -> p f c", c=NCLASS),
                    axis=mybir.AxisListType.X, op=ALU.add,
                ).then_inc(sem_red, 1)
                dve.wait_ge(sem_t, 16 * (k + 1))
                Ti = Tf2[:, h * F0:(h + 1) * F0]
                dve.tensor_copy(Ti[:], Ti2[:, h * F0:(h + 1) * F0])
                for c in range(NCLASS):
                    dve.scalar_tensor_tensor(
                        out=junk[:], in0=Ti[:], scalar=float(c),
                        in1=X2[:, h * EF0 + c * F0: h * EF0 + (c + 1) * F0],
                        op0=ALU.is_equal, op1=ALU.mult,
                        accum_out=ABN[:, 0 * SEC + k * NCLASS + c: 0 * SEC + k * NCLASS + c + 1])
                dve.wait_ge(sem_log, k + 1)
                LSE = L2[:, h * F0:(h + 1) * F0]
                for c in range(NCLASS):
                    dve.scalar_tensor_tensor(
                        out=junk[:], in0=Ti[:], scalar=float(c), in1=LSE[:],
                        op0=ALU.is_equal, op1=ALU.mult,
                        accum_out=ABN[:, 1 * SEC + k * NCLASS + c: 1 * SEC + k * NCLASS + c + 1])
                for c in range(NCLASS):
                    ins = dve.tensor_scalar(
                        out=junk[:], in0=Ti[:], scalar1=float(c), scalar2=None,
                        op0=ALU.is_equal, op1=ALU.add,
                        accum_out=ABN[:, 2 * SEC + k * NCLASS + c: 2 * SEC + k * NCLASS + c + 1])
                    if c == NCLASS - 1:
                        ins.then_inc(sem_done, 1)

        @block.tensor
        def _(pe):
            pe.wait_ge(sem_done, NCH0)
            pe.matmul(ps[:], lhsT=ones[:], rhs=ABN[:], start=True, stop=True).then_inc(sem_mm, 1)

    return nc


_CACHE = {}
_W8 = np.ascontiguousarray(
    np.broadcast_to(np.eye(P, dtype=np.float32)[:, None, :], (P, 2, P))
).reshape(P, 2 * P).astype(ml_dtypes.float8_e4m3fn)


def _weights_and_counts(target):
    t = np.asarray(target).ravel()
    valid = (t >= 0) & (t < NCLASS)
    N = np.bincount(t[valid].astype(np.int64), minlength=NCLASS).astype(np.float64)
    with np.errstate(over="ignore"):
        w = np.where(N > 0, (1.0 - BETA) / (1.0 - np.power(np.float64(BETA), N)), 0.0)
    return w, N, int(valid.sum())


def _run_fast3(logits, target, trace=False):
    if "fast3" not in _CACHE:
        _CACHE["fast3"] = _build_fast3()
    nc = _CACHE["fast3"]
    lg = np.asarray(logits)
    tg = np.asarray(target)
    in_maps = []
    for i in range(NCORES):
        xp = np.ascontiguousarray(
            lg[i].reshape(NCLASS, P, NCHUNK, F).transpose(2, 1, 0, 3)
        ).reshape(NCHUNK * P, EF).astype(ml_dtypes.bfloat16)
        in_maps.append({
            "xp": xp,
            "tgt": tg[i].reshape(P, COLS).astype(np.float16),
            "w8": _W8,
        })
    return run_bass_kernel_spmd(nc, in_maps, core_ids=list(range(NCORES)), trace=trace)


def _combine_fast3(results, n_valid):
    G1 = 0.0
    G2 = 0.0
    for i in range(NCORES):
        r = results[i]["out"].astype(np.float64).reshape(8)
        G2 += r[0:4].sum()
        G1 += r[4]
    return np.float32((G2 - G1) / n_valid)


def _run_fast(logits, target, trace=False):
    if "fast" not in _CACHE:
        _CACHE["fast"] = _build_fast()
    nc = _CACHE["fast"]
    lg = np.asarray(logits)
    tg = np.asarray(target)
    in_maps = []
    for i in range(NCORES):
        in_maps.append({
            "logits": np.ascontiguousarray(
                lg[i].reshape(NCLASS, P, COLS)).astype(ml_dtypes.bfloat16),
            "target": np.ascontiguousarray(
                tg[i].reshape(P, COLS)).astype(np.float32),
        })
    return run_bass_kernel_spmd(nc, in_maps, core_ids=list(range(NCORES)), trace=trace)


def _combine_fast(results, n_valid):
    NG = NCHUNK * NCLASS
    G1 = 0.0
    G2 = 0.0
    for i in range(NCORES):
        r = results[i]["out"].astype(np.float64).reshape(NG + NCHUNK)
        G1 += r[:NG].sum()
        G2 += r[NG:].sum()
    return np.float32((G2 - G1) / n_valid)


def _run_exact(logits, target, trace=False):
    if "exact" not in _CACHE:
        _CACHE["exact"] = _build_exact()
    nc = _CACHE["exact"]
    in_maps = []
    for i in range(NCORES):
        in_maps.append({
            "logits": np.ascontiguousarray(
                np.asarray(logits)[i].reshape(NCLASS, P, COLS)),
            "target": np.ascontiguousarray(
                np.asarray(target)[i].reshape(P, COLS)),
        })
    return run_bass_kernel_spmd(nc, in_maps, core_ids=list(range(NCORES)), trace=trace)


def _combine_exact(results, w):
    F0 = 512
    NCH0 = COLS // F0
    A = np.zeros(NCLASS, np.float64)
    B = np.zeros(NCLASS, np.float64)
    N = np.zeros(NCLASS, np.float64)
    for i in range(NCORES):
        r = results[i]["out"].astype(np.float64).reshape(3, NCH0, NCLASS).sum(axis=1)
        A += r[0]
        B += r[1]
        N += r[2]
    num = float((w * (B - A)).sum())
    den = float((w * N).sum())
    return np.float32(num / den)


def kernel(logits, target):
    assert logits.shape == (NCORES, NCLASS, 512, 1024) and logits.dtype == np.float32
    assert target.shape == (NCORES, 512, 1024) and target.dtype == np.int32
    w, N, n_valid = _weights_and_counts(target)
    pos = w[N > 0]
    equal_w = pos.size > 0 and (pos.max() - pos.min()) <= 1e-9 * pos.mean()
    if equal_w:
        if n_valid == target.size:
            r = _run_fast3(logits, target)
            return _combine_fast3(r.results, n_valid)
        r = _run_fast(logits, target)
        return _combine_fast(r.results, n_valid)
    r = _run_exact(logits, target)
    return _combine_exact(r.results, w)


# revision 26
# speedup vs baseline: 1.0882x; 1.0882x over previous
"""Class-balanced softmax cross-entropy loss on 8 Trainium2 NeuronCores.

Math: counts N_c over batch; w_c = (1-beta)/(1-beta^N_c) (0 if N_c=0);
loss = -sum w[t](logp[t]) / sum w[t] over valid pixels.

Fast path (used when all class weights are equal, which holds whenever every
class count N_c is large enough that beta^N_c underflows — always true for
this problem's 4.2M uniformly distributed pixels; verified exactly on host
via bincount): the weights cancel in the ratio, so
  loss = (sum_pix lse - sum_pix x[t]) / N_valid

fast3 (no-ignore) engine split per core (91.8us HW, vs 153.1us baseline):
  ACT : exp over all logits in DMA-slab-sized instrs (the ~65us/core floor;
        slab-granular waits keep the stream dense behind the DMA), writing
        fp8e4 E; per-chunk Ln(sumexp) from PSUM with accum_out -> Σ lse.
        A dummy activation up front pulls the ~1.3us ACT table load into
        the DMA-fill window.
  PE  : sumexp = Σ_c exp via fp8 DoubleRow matmuls (2 classes/matmul, PSUM
        accumulation; replaces the old DVE f16 tree), and Σ x[t] = Σ of
        mask*x products via ones-matmuls into one accumulating PSUM bank.
  DVE : per-class one-hot masks via tensor_scalar is_equal (4x mode) plus
        one in-place tensor_tensor product per class-group (2x mode); the
        old fused scalar_tensor_tensor path has no fast uops (1x) and was
        the 153us bottleneck. Final ps_g -> scalar reduce also on DVE.
  Out : the [128, 8] ABN accumulator (4 Ln cols + gather total) is DMA'd
        straight to DRAM; the host does the final partition reduce.
Inputs host-cast: logits -> bf16 chunk-major [4*128, 19456], target -> f16.
Pipeline: 4 col-chunks x ~5-class groups, X/E double-buffered, 4 rotating
mask buffers; chunk 0 split finer for fill, chunk 3 ends with tiny slabs
and the PE runs gathers before sums there to shorten the tail.

Exact fallback path (any weight spread): original per-class A/B/N kernel;
masked stt path for inputs with ignore pixels.
"""

import numpy as np
import sys

for _p in ("/opt/trn_rl_repo",):
    if _p not in sys.path:
        sys.path.insert(0, _p)

import ml_dtypes
from concourse import bass, mybir
from concourse.bass_utils import run_bass_kernel_spmd

NCLASS = 19
BETA = 0.999
NCORES = 8
P = 128
COLS = 4096              # 512*1024 / 128
F = 1024                 # free-dim chunk
NCHUNK = COLS // F       # 4
EF = NCLASS * F          # 19456
GROUPS = [(0, 5), (5, 10), (10, 15), (15, 19)]
GW = 5 * F               # max group width in cols

f32 = mybir.dt.float32
f16 = mybir.dt.float16
bf16 = mybir.dt.bfloat16
i32 = mybir.dt.int32
AF = mybir.ActivationFunctionType
ALU = mybir.AluOpType


def _build_fast3():
    f8 = mybir.dt.float8e4
    nc = bass.Bass()
    xp = nc.declare_dram_parameter("xp", [NCHUNK * P, EF], bf16, isOutput=False)
    tgt = nc.declare_dram_parameter("tgt", [P, COLS], f16, isOutput=False)
    w8_in = nc.declare_dram_parameter("w8", [P, 2 * P], f8, isOutput=False)
    out = nc.declare_dram_parameter("out", [P, 8], f32, isOutput=True)

    NMBUF = 4                # rotating mask/product buffers
    X2 = nc.alloc_sbuf_tensor("X2", [P, 2 * EF], bf16)
    E2 = nc.alloc_sbuf_tensor("E2", [P, 2 * EF], f8)
    T = nc.alloc_sbuf_tensor("T", [P, COLS], f16)
    M2 = nc.alloc_sbuf_tensor("M2", [P, NMBUF * GW], f16)
    junkL = nc.alloc_sbuf_tensor("junkL", [P, F], f16)
    ABN = nc.alloc_sbuf_tensor("ABN", [P, 8], f32)
    W8 = nc.alloc_sbuf_tensor("W8", [P, 2 * P], f8)
    ones_g = nc.alloc_sbuf_tensor("ones_g", [P, 1], f16)
    junkG = nc.alloc_sbuf_tensor("junkG", [1, 512], f16)
    ps_s = nc.alloc_psum_tensor("ps_s", [P, 2 * F], f32)
    ps_g = nc.alloc_psum_tensor("ps_g", [1, 512], f32)

    # sumexp class pairs (DoubleRow sums 2 classes/matmul); grouped so waits
    # track exp progress
    PAIR_GROUPS = [[(0, 1), (2, 3)], [(4, 5), (6, 7), (8, 9)],
                   [(10, 11), (12, 13)], [(14, 15), (16, 17)], [(18, None)]]
    # per-chunk X sub-DMA class splits (chunk 0 finer for fill, chunk 3 with a
    # tiny last slab for the tail); exp instrs mirror slabs except middle
    # chunks which batch up
    DMA_SPLITS = [[(0, 1), (1, 3), (3, 5), (5, 10), (10, 15), (15, 19)],
                  [(0, 5), (5, 10), (10, 15), (15, 19)],
                  [(0, 5), (5, 10), (10, 15), (15, 19)],
                  [(0, 5), (5, 10), (10, 15), (15, 18), (18, 19)]]
    EXP_SPLITS = [DMA_SPLITS[0],
                  [(0, 10), (10, 19)], [(0, 10), (10, 19)],
                  DMA_SPLITS[3]]
    # per-chunk mask/product groups (chunk 3 ends with a 2-class group)
    MGROUPS = [GROUPS, GROUPS, GROUPS,
               [(0, 5), (5, 10), (10, 15), (15, 17), (17, 19)]]
    mg_cum = [0]
    for k in range(NCHUNK):
        mg_cum.append(mg_cum[-1] + len(MGROUPS[k]))
    NMG = mg_cum[-1]

    dma_done_at = []
    n = 0
    for k in range(NCHUNK):
        ends = {}
        for (lo, hi) in DMA_SPLITS[k]:
            n += 1
            ends[hi] = n
        dma_done_at.append(ends)

    def xdma_thr(k, hi):
        ends = dma_done_at[k]
        best = min(e for e in ends if e >= hi)
        return 16 * ends[best]

    def tt_thr(k, hi):
        """s_tt threshold for products covering classes [0, hi) of chunk k."""
        ng = 0
        for (lo2, hi2) in MGROUPS[k]:
            ng += 1
            if hi2 >= hi:
                break
        return mg_cum[k] + ng

    with (
        nc.Block() as block,
        nc.semaphore("s_t") as s_t,
        nc.semaphore("s_id") as s_id,
        nc.semaphore("s_x") as s_x,
        nc.semaphore("s_e") as s_e,      # counts classes exp'd: 19*k + hi
        nc.semaphore("s_tt") as s_tt,
        nc.semaphore("s_ps") as s_ps,
        nc.semaphore("s_pg") as s_pg,
        nc.semaphore("s_ln") as s_ln,
        nc.semaphore("s_gs") as s_gs,
        nc.semaphore("s_out") as s_out,
        nc.allow_low_precision("f16 masks/products; fp8 exp; f32 accum"),
    ):
        @block.sync
        def _(sp):
            for k in range(NCHUNK):
                h = k % 2
                for j, (lo, hi) in enumerate(DMA_SPLITS[k]):
                    if k >= 2:
                        # X half reused: chunk k-2's exp + products done
                        sp.wait_ge(s_e, 19 * (k - 2) + hi)
                        sp.wait_ge(s_tt, tt_thr(k - 2, hi))
                    sp.dma_start(
                        X2[:, h * EF + lo * F: h * EF + hi * F],
                        xp[k * P:(k + 1) * P, lo * F: hi * F],
                    ).then_inc(s_x, 16)
                    if k == 0 and j == 1:
                        sp.dma_start(T[:, 0:F], tgt[:, 0:F]).then_inc(s_t, 16)
                        sp.dma_start(W8[:], w8_in[:, :]).then_inc(s_id, 16)
                    elif k >= 1 and j == 0:
                        sp.dma_start(T[:, k * F:(k + 1) * F],
                                     tgt[:, k * F:(k + 1) * F]).then_inc(s_t, 16)

        @block.scalar
        def _(act):
            def ln_chunk(kk):
                hh = kk % 2
                act.wait_ge(s_ps, kk + 1)
                act.activation(
                    junkL[:], ps_s[:, hh * F:(hh + 1) * F], AF.Ln,
                    accum_out=ABN[:, kk:kk + 1]).then_inc(s_ln, 1)

            # dummy activation: pulls the ACT table load into the DMA-fill
            # window instead of serializing it before the first real exp
            act.activation(junkL[:, 0:16], junkL[:, 16:32], AF.Exp)
            for k in range(NCHUNK):
                h = k % 2
                ln_at = 0 if len(EXP_SPLITS[k]) == 2 else 1
                for j, (lo, hi) in enumerate(EXP_SPLITS[k]):
                    act.wait_ge(s_x, xdma_thr(k, hi))
                    if k >= 2 and j == 0:
                        act.wait_ge(s_ps, k - 1)   # E half reused
                    act.activation(
                        E2[:, h * EF + lo * F: h * EF + hi * F],
                        X2[:, h * EF + lo * F: h * EF + hi * F],
                        AF.Exp).then_inc(s_e, hi - lo)
                    if k >= 1 and j == ln_at:
                        ln_chunk(k - 1)
            ln_chunk(NCHUNK - 1)
            # tail: ABN (Ln cols + gather total) straight to dram; host reduces
            act.wait_ge(s_gs, 1)
            act.dma_start(out[:, :], ABN[:]).then_inc(s_out, 16)
            act.wait_ge(s_out, 16)

        @block.vector
        def _(dve):
            dve.memset(ABN[:], 0.0)
            dve.memset(ones_g[:], 1.0)
            for k in range(NCHUNK):
                h = k % 2
                dve.wait_ge(s_t, 16 * (k + 1))
                Tk = T[:, k * F:(k + 1) * F]
                for g, (lo, hi) in enumerate(MGROUPS[k]):
                    G = mg_cum[k] + g
                    hm = G % NMBUF
                    W = (hi - lo) * F
                    Mg = M2[:, hm * GW: hm * GW + W]
                    if G >= NMBUF:
                        dve.wait_ge(s_pg, G - (NMBUF - 1))   # M buf reused
                    for ci, c in enumerate(range(lo, hi)):
                        dve.tensor_scalar(
                            out=Mg[:, ci * F:(ci + 1) * F], in0=Tk,
                            scalar1=float(c), scalar2=None, op0=ALU.is_equal)
                    dve.wait_ge(s_x, xdma_thr(k, hi))
                    dve.tensor_tensor(
                        out=Mg[:], in0=Mg[:],
                        in1=X2[:, h * EF + lo * F: h * EF + hi * F],
                        op=ALU.mult).then_inc(s_tt, 1)
            # total gather sum: reduce ps_g into ABN[0, 4] (rows 1.. are 0)
            dve.wait_ge(s_pg, NMG)
            dve.tensor_scalar(
                out=junkG[:], in0=ps_g[:, :], scalar1=1.0, scalar2=None,
                op0=ALU.mult, op1=ALU.add,
                accum_out=ABN[0:1, 4:5]).then_inc(s_gs, 1)

        @block.tensor
        def _(pe):
            pe.wait_ge(s_id, 16)
            w_dr = W8[:].rearrange("p (two m) -> p two m", two=2)
            first_g = True

            def sum_pairs(k, pg):
                h = k % 2
                for (c0, c1) in PAIR_GROUPS[pg]:
                    pe.wait_ge(s_e, 19 * k + (c1 if c1 is not None else c0) + 1)
                    for half in range(2):
                        dst = ps_s[:, h * F + half * 512: h * F + (half + 1) * 512]
                        base = h * EF + c0 * F + half * 512
                        if c1 is not None:
                            rhs = E2[:, base: base + 2 * F].rearrange(
                                "p (two f) -> p two f", two=2)[:, :, 0:512]
                            pe.matmul(
                                dst, lhsT=w_dr, rhs=rhs,
                                perf_mode=mybir.MatmulPerfMode.DoubleRow,
                                start=(c0 == 0), stop=False,
                                skip_group_check=True)
                        else:
                            ins = pe.matmul(
                                dst, lhsT=W8[:, 0:P],
                                rhs=E2[:, base: base + 512],
                                start=False, stop=True,
                                skip_group_check=True)
                            if half == 1:
                                ins.then_inc(s_ps, 1)

            def gather(k, g):
                nonlocal first_g
                lo, hi = MGROUPS[k][g]
                G = mg_cum[k] + g
                hm = G % NMBUF
                pe.wait_ge(s_tt, G + 1)
                n = (hi - lo) * 2
                for j in range(n):
                    ins = pe.matmul(
                        ps_g[:, :], lhsT=ones_g[:],
                        rhs=M2[:, hm * GW + j * 512: hm * GW + (j + 1) * 512],
                        start=first_g, stop=(G == NMG - 1 and j == n - 1),
                        skip_group_check=True)
                    first_g = False
                    if j == n - 1:
                        ins.then_inc(s_pg, 1)

            for k in range(NCHUNK):
                if k >= 2:
                    pe.wait_ge(s_ln, k - 1)        # psum half reused
                if k < NCHUNK - 1:
                    npg = len(PAIR_GROUPS)
                    nmg = len(MGROUPS[k])
                    for i in range(max(npg, nmg)):
                        if i < npg:
                            sum_pairs(k, i)
                        if i < nmg:
                            gather(k, i)
                else:
                    order = [("g", 0), ("s", 0), ("g", 1), ("s", 1),
                             ("g", 2), ("s", 2), ("g", 3), ("s", 3),
                             ("g", 4), ("s", 4)]
                    for kind, i in order:
                        if kind == "g":
                            gather(k, i)
                        else:
                            sum_pairs(k, i)

    return nc


def _build_fast():
    """Masked fast path (handles ignore pixels); original stt kernel."""
    nc = bass.Bass()
    logits = nc.declare_dram_parameter("logits", [NCLASS, P, COLS], bf16, isOutput=False)
    target = nc.declare_dram_parameter("target", [P, COLS], f32, isOutput=False)
    NG = NCHUNK * NCLASS
    ACC_COLS = NG + NCHUNK
    out = nc.declare_dram_parameter("out", [1, ACC_COLS], f32, isOutput=True)

    X2 = nc.alloc_sbuf_tensor("X2", [P, 2 * EF], bf16)
    E2 = nc.alloc_sbuf_tensor("E2", [P, 2 * EF], f16)
    T = nc.alloc_sbuf_tensor("T", [P, COLS], f32)
    L2 = nc.alloc_sbuf_tensor("L2", [P, 2 * F], f16)
    junk = nc.alloc_sbuf_tensor("junk", [P, F], f16)
    ABN = nc.alloc_sbuf_tensor("ABN", [P, ACC_COLS], f32)
    ones = nc.alloc_sbuf_tensor("ones", [P, 1], f32)
    res = nc.alloc_sbuf_tensor("res", [1, ACC_COLS], f32)
    ps = nc.alloc_psum_tensor("ps", [1, ACC_COLS], f32)

    with (
        nc.Block() as block,
        nc.semaphore("s_x") as s_x,
        nc.semaphore("s_t") as s_t,
        nc.semaphore("s_exp") as s_exp,
        nc.semaphore("s_tree") as s_tree,
        nc.semaphore("s_log") as s_log,
        nc.semaphore("s_gA") as s_gA,
        nc.semaphore("s_gB") as s_gB,
        nc.semaphore("s_mm") as s_mm,
        nc.semaphore("s_out") as s_out,
        nc.allow_low_precision("f16 tree-sum of exp; error ~0.1% on lse"),
    ):
        @block.sync
        def _(sp):
            sp.dma_start(T[:], target[:, :]).then_inc(s_t, 16)
            for k in range(NCHUNK):
                h = k % 2
                if k >= 2:
                    sp.wait_ge(s_gA, k - 1)
                sp.dma_start(
                    X2[:, h * EF:(h + 1) * EF].rearrange("p (c f) -> p c f", c=NCLASS),
                    logits[:, :, k * F:(k + 1) * F].rearrange("c p f -> p c f"),
                ).then_inc(s_x, 16)

        @block.scalar
        def _(act):
            for k in range(NCHUNK):
                h = k % 2
                act.wait_ge(s_x, 16 * (k + 1))
                E = E2[:, h * EF:(h + 1) * EF]
                X = X2[:, h * EF:(h + 1) * EF]
                for c in range(NCLASS):
                    ins = act.activation(
                        E[:, c * F:(c + 1) * F], X[:, c * F:(c + 1) * F], AF.Exp)
                    if c == NCLASS - 1:
                        ins.then_inc(s_exp, 1)
                act.wait_ge(s_tree, k + 1)
                if k >= 2:
                    act.wait_ge(s_gB, k - 1)   # L half reused
                act.activation(
                    L2[:, h * F:(h + 1) * F], E[:, 0:F], AF.Ln,
                ).then_inc(s_log, 1)
            act.wait_ge(s_mm, 1)
            act.copy(res[:], ps[:])
            act.dma_start(out[:, :], res[:]).then_inc(s_out, 16)
            act.wait_ge(s_out, 16)

        @block.vector
        def _(dve):
            dve.memset(ABN[:], 0.0)
            dve.memset(ones[:], 1.0)
            dve.wait_ge(s_t, 16)
            for k in range(NCHUNK):
                h = k % 2
                dve.wait_ge(s_exp, k + 1)
                E = E2[:, h * EF:(h + 1) * EF]
                dve.tensor_tensor(out=E[:, 0:3 * F], in0=E[:, 0:3 * F],
                                  in1=E[:, 16 * F:19 * F], op=ALU.add)
                dve.tensor_tensor(out=E[:, 0:8 * F], in0=E[:, 0:8 * F],
                                  in1=E[:, 8 * F:16 * F], op=ALU.add)
                dve.tensor_tensor(out=E[:, 0:4 * F], in0=E[:, 0:4 * F],
                                  in1=E[:, 4 * F:8 * F], op=ALU.add)
                dve.tensor_tensor(out=E[:, 0:2 * F], in0=E[:, 0:2 * F],
                                  in1=E[:, 2 * F:4 * F], op=ALU.add)
                dve.tensor_tensor(out=E[:, 0:F], in0=E[:, 0:F],
                                  in1=E[:, F:2 * F], op=ALU.add).then_inc(s_tree, 1)
                X = X2[:, h * EF:(h + 1) * EF]
                Tk = T[:, k * F:(k + 1) * F]
                for c in range(NCLASS):
                    ins = dve.scalar_tensor_tensor(
                        out=junk[:], in0=Tk, scalar=float(c),
                        in1=X[:, c * F:(c + 1) * F],
                        op0=ALU.is_equal, op1=ALU.mult,
                        accum_out=ABN[:, k * NCLASS + c: k * NCLASS + c + 1])
                    if c == NCLASS - 1:
                        ins.then_inc(s_gA, 1)
                dve.wait_ge(s_log, k + 1)
                dve.scalar_tensor_tensor(
                    out=junk[:], in0=Tk, scalar=-0.5,
                    in1=L2[:, h * F:(h + 1) * F],
                    op0=ALU.is_gt, op1=ALU.mult,
                    accum_out=ABN[:, NG + k: NG + k + 1]).then_inc(s_gB, 1)

        @block.tensor
        def _(pe):
            pe.wait_ge(s_gB, NCHUNK)
            pe.matmul(ps[:], lhsT=ones[:], rhs=ABN[:], start=True, stop=True
                      ).then_inc(s_mm, 1)

    return nc


def _build_exact():
    """Original per-class A/B/N kernel (correct for any weight pattern)."""
    nc = bass.Bass()
    F0 = 512
    NCH0 = COLS // F0
    SEC = NCH0 * NCLASS
    ACC0 = 3 * SEC
    logits = nc.declare_dram_parameter("logits", [NCLASS, P, COLS], f32, isOutput=False)
    target = nc.declare_dram_parameter("target", [P, COLS], i32, isOutput=False)
    out = nc.declare_dram_parameter("out", [1, ACC0], f32, isOutput=True)

    EF0 = NCLASS * F0
    X2 = nc.alloc_sbuf_tensor("X2", [P, 2 * EF0], f32)
    E2 = nc.alloc_sbuf_tensor("E2", [P, 2 * EF0], f32)
    Ti2 = nc.alloc_sbuf_tensor("Ti2", [P, 2 * F0], i32)
    Tf2 = nc.alloc_sbuf_tensor("Tf2", [P, 2 * F0], f32)
    S2 = nc.alloc_sbuf_tensor("S2", [P, 2 * F0], f32)
    L2 = nc.alloc_sbuf_tensor("L2", [P, 2 * F0], f32)
    junk = nc.alloc_sbuf_tensor("junk", [P, F0], f32)
    ABN = nc.alloc_sbuf_tensor("ABN", [P, ACC0], f32)
    ones = nc.alloc_sbuf_tensor("ones", [P, 1], f32)
    res = nc.alloc_sbuf_tensor("res", [1, ACC0], f32)
    ps = nc.alloc_psum_tensor("ps", [1, ACC0], f32)

    with (
        nc.Block() as block,
        nc.semaphore("sem_x") as sem_x,
        nc.semaphore("sem_t") as sem_t,
        nc.semaphore("sem_exp") as sem_exp,
        nc.semaphore("sem_red") as sem_red,
        nc.semaphore("sem_log") as sem_log,
        nc.semaphore("sem_done") as sem_done,
        nc.semaphore("sem_mm") as sem_mm,
        nc.semaphore("sem_out") as sem_out,
    ):
        @block.scalar
        def _(act):
            for k in range(NCH0):
                h = k % 2
                if k >= 2:
                    act.wait_ge(sem_done, k - 1)
                act.dma_start(
                    X2[:, h * EF0:(h + 1) * EF0].rearrange("p (c f) -> p c f", c=NCLASS),
                    logits[:, :, k * F0:(k + 1) * F0].rearrange("c p f -> p c f"),
                ).then_inc(sem_x, 16)
                act.dma_start(
                    Ti2[:, h * F0:(h + 1) * F0], target[:, k * F0:(k + 1) * F0],
                ).then_inc(sem_t, 16)
                act.wait_ge(sem_x, 16 * (k + 1))
                for c in range(NCLASS):
                    ins = act.activation(
                        E2[:, h * EF0 + c * F0: h * EF0 + (c + 1) * F0],
                        X2[:, h * EF0 + c * F0: h * EF0 + (c + 1) * F0], AF.Exp)
                    if c == NCLASS - 1:
                        ins.then_inc(sem_exp, 1)
                act.wait_ge(sem_red, k + 1)
                act.activation(
                    L2[:, h * F0:(h + 1) * F0], S2[:, h * F0:(h + 1) * F0], AF.Ln,
                ).then_inc(sem_log, 1)
            act.wait_ge(sem_mm, 1)
            act.copy(res[:], ps[:])
            act.dma_start(out[:, :], res[:]).then_inc(sem_out, 16)
            act.wait_ge(sem_out, 16)

        @block.vector
        def _(dve):
            dve.memset(ABN[:], 0.0)
            dve.memset(ones[:], 1.0)
            for k in range(NCH0):
                h = k % 2
                dve.wait_ge(sem_exp, k + 1)
                dve.tensor_reduce(
                    S2[:, h * F0:(h + 1) * F0],
                    E2[:, h * EF0:(h + 1) * EF0].rearrange("p (c f) -> p f c", c=NCLASS),
                    axis=mybir.AxisListType.X, op=ALU.add,
                ).then_inc(sem_red, 1)
                dve.wait_ge(sem_t, 16 * (k + 1))
                Ti = Tf2[:, h * F0:(h + 1) * F0]
                dve.tensor_copy(Ti[:], Ti2[:, h * F0:(h + 1) * F0])
                for c in range(NCLASS):
                    dve.scalar_tensor_tensor(
                        out=junk[:], in0=Ti[:], scalar=float(c),
                        in1=X2[:, h * EF0 + c * F0: h * EF0 + (c + 1) * F0],
                        op0=ALU.is_equal, op1=ALU.mult,
                        accum_out=ABN[:, 0 * SEC + k * NCLASS + c: 0 * SEC + k * NCLASS + c + 1])
                dve.wait_ge(sem_log, k + 1)
                LSE = L2[:, h * F0:(h + 1) * F0]
                for c in range(NCLASS):
                    dve.scalar_tensor_tensor(
                        out=junk[:], in0=Ti[:], scalar=float(c), in1=LSE[:],
                        op0=ALU.is_equal, op1=ALU.mult,
                        accum_out=ABN[:, 1 * SEC + k * NCLASS + c: 1 * SEC + k * NCLASS + c + 1])
                for c in range(NCLASS):
                    ins = dve.tensor_scalar(
                        out=junk[:], in0=Ti[:], scalar1=float(c), scalar2=None,
                        op0=ALU.is_equal, op1=ALU.add,
                        accum_out=ABN[:, 2 * SEC + k * NCLASS + c: 2 * SEC + k * NCLASS + c + 1])
                    if c == NCLASS - 1:
                        ins.then_inc(sem_done, 1)

        @block.tensor
        def _(pe):
            pe.wait_ge(sem_done, NCH0)
            pe.matmul(ps[:], lhsT=ones[:], rhs=ABN[:], start=True, stop=True).then_inc(sem_mm, 1)

    return nc


_CACHE = {}
_W8 = np.ascontiguousarray(
    np.broadcast_to(np.eye(P, dtype=np.float32)[:, None, :], (P, 2, P))
).reshape(P, 2 * P).astype(ml_dtypes.float8_e4m3fn)


def _weights_and_counts(target):
    t = np.asarray(target).ravel()
    valid = (t >= 0) & (t < NCLASS)
    N = np.bincount(t[valid].astype(np.int64), minlength=NCLASS).astype(np.float64)
    with np.errstate(over="ignore"):
        w = np.where(N > 0, (1.0 - BETA) / (1.0 - np.power(np.float64(BETA), N)), 0.0)
    return w, N, int(valid.sum())


def _run_fast3(logits, target, trace=False):
    if "fast3" not in _CACHE:
        _CACHE["fast3"] = _build_fast3()
    nc = _CACHE["fast3"]
    lg = np.asarray(logits)
    tg = np.asarray(target)
    in_maps = []
    for i in range(NCORES):
        xp = np.ascontiguousarray(
            lg[i].reshape(NCLASS, P, NCHUNK, F).transpose(2, 1, 0, 3)
        ).reshape(NCHUNK * P, EF).astype(ml_dtypes.bfloat16)
        in_maps.append({
            "xp": xp,
            "tgt": tg[i].reshape(P, COLS).astype(np.float16),
            "w8": _W8,
        })
    return run_bass_kernel_spmd(nc, in_maps, core_ids=list(range(NCORES)), trace=trace)


def _combine_fast3(results, n_valid):
    G1 = 0.0
    G2 = 0.0
    for i in range(NCORES):
        r = results[i]["out"].astype(np.float64).reshape(P, 8)
        G2 += r[:, 0:4].sum()
        G1 += r[:, 4].sum()
    return np.float32((G2 - G1) / n_valid)


def _run_fast(logits, target, trace=False):
    if "fast" not in _CACHE:
        _CACHE["fast"] = _build_fast()
    nc = _CACHE["fast"]
    lg = np.asarray(logits)
    tg = np.asarray(target)
    in_maps = []
    for i in range(NCORES):
        in_maps.append({
            "logits": np.ascontiguousarray(
                lg[i].reshape(NCLASS, P, COLS)).astype(ml_dtypes.bfloat16),
            "target": np.ascontiguousarray(
                tg[i].reshape(P, COLS)).astype(np.float32),
        })
    return run_bass_kernel_spmd(nc, in_maps, core_ids=list(range(NCORES)), trace=trace)


def _combine_fast(results, n_valid):
    NG = NCHUNK * NCLASS
    G1 = 0.0
    G2 = 0.0
    for i in range(NCORES):
        r = results[i]["out"].astype(np.float64).reshape(NG + NCHUNK)
        G1 += r[:NG].sum()
        G2 += r[NG:].sum()
    return np.float32((G2 - G1) / n_valid)


def _run_exact(logits, target, trace=False):
    if "exact" not in _CACHE:
        _CACHE["exact"] = _build_exact()
    nc = _CACHE["exact"]
    in_maps = []
    for i in range(NCORES):
        in_maps.append({
            "logits": np.ascontiguousarray(
                np.asarray(logits)[i].reshape(NCLASS, P, COLS)),
            "target": np.ascontiguousarray(
                np.asarray(target)[i].reshape(P, COLS)),
        })
    return run_bass_kernel_spmd(nc, in_maps, core_ids=list(range(NCORES)), trace=trace)


def _combine_exact(results, w):
    F0 = 512
    NCH0 = COLS // F0
    A = np.zeros(NCLASS, np.float64)
    B = np.zeros(NCLASS, np.float64)
    N = np.zeros(NCLASS, np.float64)
    for i in range(NCORES):
        r = results[i]["out"].astype(np.float64).reshape(3, NCH0, NCLASS).sum(axis=1)
        A += r[0]
        B += r[1]
        N += r[2]
    num = float((w * (B - A)).sum())
    den = float((w * N).sum())
    return np.float32(num / den)


def kernel(logits, target):
    assert logits.shape == (NCORES, NCLASS, 512, 1024) and logits.dtype == np.float32
    assert target.shape == (NCORES, 512, 1024) and target.dtype == np.int32
    w, N, n_valid = _weights_and_counts(target)
    pos = w[N > 0]
    equal_w = pos.size > 0 and (pos.max() - pos.min()) <= 1e-9 * pos.mean()
    if equal_w:
        if n_valid == target.size:
            r = _run_fast3(logits, target)
            return _combine_fast3(r.results, n_valid)
        r = _run_fast(logits, target)
        return _combine_fast(r.results, n_valid)
    r = _run_exact(logits, target)
    return _combine_exact(r.results, w)
